# revision 17
# baseline (speedup 1.0000x reference)
"""Bass/Trainium2 kernel for nn_BinaryResNetBlock (bireal block, stride 1).

Computation (reference):
    stage(x, W, g, b): a = sign(x); wb = mean(|W|)*sign(W)
                       y = conv3x3(a, wb, pad=1); BN(train-mode, batch stats)
    inner = stage(x, W1, g1, b1) + x
    out   = stage(inner, W2, g2, b2) + inner

Strategy:
  - Data parallel over batch: N=32 -> 4 images per core on 8 cores.
  - conv(sign(x), sign(W)) accumulates exact small integers in fp32 PSUM, so
    fp8(e4m3) matmuls in DoubleRow mode (K=256 per MM) are bit-exact.
    Conv outputs stored as int16 (|c| <= 2304).
  - BN batch stats: bn_stats/bn_aggr per core -> (S1, S2) per channel ->
    2KB AllReduce across the 8 cores -> per-channel affine A, B on chip.
  - BN apply + skip-add fused: t = c*A + x (scalar_tensor_tensor on DVE),
    sign2 = Sign(t + B) on ACT with per-partition bias.
  - Final: out = c2*A2 + (c1*A1 + x) + (B1 + B2), computed in-place in the
    streamed x tile; x is streamed from HBM three times to keep SBUF small.
"""

import os
import sys

import numpy as np


def _ensure_path():
    try:
        import concourse.bass  # noqa: F401
    except ImportError:
        for p in ("/opt/trn_rl_repo", "/root/.axon_site/_ro/trn_rl_repo"):
            if os.path.isdir(p) and p not in sys.path:
                sys.path.insert(0, p)


_ensure_path()

import ml_dtypes  # noqa: E402

import concourse.bacc as bacc  # noqa: E402
import concourse.bass as bass  # noqa: E402
import concourse.mybir as mybir  # noqa: E402
import concourse.tile as tile  # noqa: E402
from concourse import bass_utils  # noqa: E402

F32 = mybir.dt.float32
I16 = mybir.dt.int16
F8 = mybir.dt.float8e4
F16 = mybir.dt.float16
BF16 = mybir.dt.bfloat16
NP_F8 = ml_dtypes.float8_e4m3
NP_BF16 = ml_dtypes.bfloat16

C = 256  # channels
P = 128  # partitions
NCH = C // P  # channel chunks (2)
WID = 56  # image width (fixed)
PW = WID + 2  # padded width (58)
RB = 8  # output rows per PSUM tile
EPS = 1e-5

# module-level knobs (test.py may set these)
TRACE = False
TRACE_KW = {}

Alu = mybir.AluOpType
Act = mybir.ActivationFunctionType


def build_nc(n_img, h, n_cores, use_dr=True):
    """Build the SPMD Bass program (same on every core)."""
    assert h % RB == 0
    nrb = h // RB
    ph = h + 2
    plane = ph * PW
    pstride = (plane + 15) // 16 * 16  # DoubleRow needs 16B-aligned k-step
    hw = h * WID
    free = RB * PW  # matmul free dim (464); cols w=56,57 of each row are junk
    m_loc = n_img * hw
    m_glob = n_cores * m_loc
    adt = F8 if use_dr else BF16

    nc = bacc.Bacc(
        "TRN2", target_bir_lowering=False, debug=False, num_devices=n_cores
    )
    x_d = nc.dram_tensor("x", [n_img, C, h, WID], F32, kind="ExternalInput").ap()
    w_d = [
        nc.dram_tensor(f"wb{s + 1}", [P, 9, NCH, C], adt, kind="ExternalInput").ap()
        for s in range(2)
    ]
    # coefs[:, ch, k]: k=0 gamma1*scale1, 1 beta1, 2 gamma2*scale2, 3 beta2,
    #                 4 scale1^2 (bcast), 5 scale2^2 (bcast)
    cf_d = nc.dram_tensor("coefs", [P, NCH, 6], F32, kind="ExternalInput").ap()
    out_d = nc.dram_tensor("out", [n_img, C, h, WID], F32, kind="ExternalOutput").ap()

    with tile.TileContext(nc) as tc:
        with (
            tc.tile_pool(name="persist", bufs=1) as persist,
            tc.tile_pool(name="abuf", bufs=1) as abuf,
            tc.tile_pool(name="cbuf", bufs=1) as cbuf,
            tc.tile_pool(name="statsp", bufs=1) as statsp,
            tc.tile_pool(name="xs", bufs=4) as xs,
            tc.tile_pool(name="small", bufs=2) as small,
            tc.tile_pool(name="ps", bufs=8, space="PSUM") as psp,
            tc.tile_pool(name="dram", bufs=2, space="DRAM") as dramp,
        ):
            # ---- persistent tiles ----
            w_t = []
            for s in range(2):
                wt = persist.tile([P, 9, NCH, C], adt, tag=f"w{s}")
                nc.sync.dma_start(out=wt, in_=w_d[s])
                w_t.append(wt)
            coefs = persist.tile([P, NCH, 6], F32, tag="coefs")
            nc.sync.dma_start(out=coefs, in_=cf_d)
            stt1 = persist.tile([P, NCH, 6], F32, tag="stt1")
            eps_t = persist.tile([P, 1], F32, tag="eps")
            nc.vector.memset(eps_t, EPS)
            stt2 = persist.tile([P, NCH, 6], F32, tag="stt2")
            b12 = persist.tile([P, NCH, 1], F32, tag="b12")

            c_t = [
                {
                    (i, ch): cbuf.tile(
                        [P, hw], I16, tag=f"c{s}_{i}_{ch}", name=f"c{s}_{i}_{ch}"
                    )
                    for i in range(n_img)
                    for ch in range(NCH)
                }
                for s in range(2)
            ]


            def zero_borders(a_t, ch):
                """Zero the pad borders + tail pad of one padded plane."""
                pl = a_t[:, ch, 0:plane].rearrange("p (r c) -> p r c", c=PW)
                nc.gpsimd.memset(pl[:, 0:1, :], 0.0)
                nc.gpsimd.memset(pl[:, h + 1 : h + 2, :], 0.0)
                nc.gpsimd.memset(pl[:, 1 : h + 1, 0:1], 0.0)
                nc.gpsimd.memset(pl[:, 1 : h + 1, PW - 1 : PW], 0.0)
                if pstride > plane:
                    nc.gpsimd.memset(a_t[:, ch, plane:pstride], 0.0)

            def sign_view(a_t, ch):
                """Interior [P, h, WID] view of the padded plane (rows/cols 1..)."""
                return a_t[:, ch, 0:plane].rearrange(
                    "p (r c) -> p r c", c=PW
                )[:, 1 : h + 1, 1 : WID + 1]

            def conv_one_img(wt, a_t, i, c_tile, stats_t):  # a_t: per-img tile
                """3x3 binary conv for image i (both out-chunks) + psum drains.

                Taps outer over a block of PSUM tiles -> each LDWEIGHTS is
                reused across the block."""
                for ch_o in range(NCH):
                    for rb in range(nrb):
                        ps = psp.tile([P, free], F32, tag="ps", name="ps")
                        for tap in range(9):
                            dh, dw = divmod(tap, 3)
                            off = (rb * RB + dh) * PW + dw
                            if use_dr:
                                nc.tensor.matmul(
                                    ps,
                                    wt[:, tap, :, ch_o * P : (ch_o + 1) * P],
                                    a_t[:, 0:2, off : off + free],
                                    start=(tap == 0),
                                    stop=(tap == 8),
                                    perf_mode=mybir.MatmulPerfMode.DoubleRow,
                                )
                            else:
                                for kc in range(NCH):
                                    nc.tensor.matmul(
                                        ps,
                                        wt[:, tap, kc, ch_o * P : (ch_o + 1) * P],
                                        a_t[:, kc, off : off + free],
                                        start=(tap == 0 and kc == 0),
                                        stop=(tap == 8 and kc == NCH - 1),
                                    )
                        pv = ps.rearrange("p (r c) -> p r c", c=PW)[:, :, 0:WID]
                        cs = c_tile[(i, ch_o)][
                            :, rb * RB * WID : (rb + 1) * RB * WID
                        ]
                        nc.scalar.copy(
                            out=cs.rearrange("p (r c) -> p r c", c=WID),
                            in_=pv,
                        )
                        nc.vector.bn_stats(
                            out=stats_t[:, ch_o, i * nrb + rb], in_=cs
                        )

            def bn_coeffs(stats_t, gs_col, b_col, ssq_col, stt):
                """Aggregate local stats, AllReduce, compute per-channel A, B.

                stt columns: 0 mu_c, 1 var_c, 2 inv, 3 A, 4 B, 5 tmp
                """
                mv = small.tile([P, NCH, 2], F32, tag="mv")
                for ch in range(NCH):
                    nc.vector.bn_aggr(out=mv[:, ch], in_=stats_t[:, ch])
                ar = small.tile([P, NCH, 2], F32, tag="ar")
                tmp = small.tile([P, NCH, 1], F32, tag="tmp")
                # S1 = mean * m_loc ; S2 = (var + mean^2) * m_loc
                nc.vector.tensor_scalar(
                    out=ar[:, :, 0:1], in0=mv[:, :, 0:1],
                    scalar1=float(m_loc), scalar2=None, op0=Alu.mult,
                )
                nc.vector.tensor_mul(tmp, mv[:, :, 0:1], mv[:, :, 0:1])
                nc.vector.tensor_add(tmp, tmp, mv[:, :, 1:2])
                nc.vector.tensor_scalar(
                    out=ar[:, :, 1:2], in0=tmp,
                    scalar1=float(m_loc), scalar2=None, op0=Alu.mult,
                )
                d_in = dramp.tile([P, NCH * 2], F32, tag="d_in")
                d_out = dramp.tile(
                    [n_cores * P, NCH * 2], F32, tag="d_out",
                    addr_space="Shared" if n_cores > 4 else "Local",
                )
                nc.gpsimd.dma_start(out=d_in, in_=ar.rearrange("p a b -> p (a b)"))
                nc.gpsimd.collective_compute(
                    "AllGather",
                    Alu.bypass,
                    replica_groups=[list(range(n_cores))],
                    ins=[d_in.opt()],
                    outs=[d_out.opt()],
                )
                g_all = small.tile([P, n_cores, NCH * 2], F32, tag="g_all")
                nc.gpsimd.dma_start(
                    out=g_all, in_=d_out.rearrange("(r p) f -> p r f", p=P)
                )
                g = small.tile([P, NCH, 2], F32, tag="g")
                # sum over the rank axis (view [P, f, r], reduce innermost)
                nc.vector.tensor_reduce(
                    out=g.rearrange("p a b -> p (a b)"),
                    in_=g_all.rearrange("p r f -> p f r"),
                    axis=mybir.AxisListType.X,
                    op=Alu.add,
                )
                inv_m = float(1.0 / m_glob)
                nc.vector.tensor_scalar(
                    out=stt[:, :, 0:1], in0=g[:, :, 0:1],
                    scalar1=inv_m, scalar2=None, op0=Alu.mult,
                )
                nc.vector.tensor_scalar(
                    out=stt[:, :, 1:2], in0=g[:, :, 1:2],
                    scalar1=inv_m, scalar2=None, op0=Alu.mult,
                )
                nc.vector.tensor_mul(stt[:, :, 5:6], stt[:, :, 0:1], stt[:, :, 0:1])
                nc.vector.tensor_sub(stt[:, :, 1:2], stt[:, :, 1:2], stt[:, :, 5:6])
                # sd = sqrt(var_c * scale^2 + eps); inv = 1/sd
                nc.scalar.activation(
                    out=stt[:, :, 2:3], in_=stt[:, :, 1:2], func=Act.Sqrt,
                    bias=eps_t, scale=coefs[:, 0, ssq_col : ssq_col + 1],
                )
                nc.vector.reciprocal(out=stt[:, :, 2:3], in_=stt[:, :, 2:3])
                # A = inv * (gamma*scale);  B = beta - mu_c * A
                nc.vector.tensor_mul(
                    stt[:, :, 3:4], stt[:, :, 2:3], coefs[:, :, gs_col : gs_col + 1]
                )
                nc.vector.tensor_mul(stt[:, :, 5:6], stt[:, :, 0:1], stt[:, :, 3:4])
                nc.vector.tensor_sub(
                    stt[:, :, 4:5], coefs[:, :, b_col : b_col + 1], stt[:, :, 5:6]
                )

            # ================= stage 1 =================
            a_ts = [
                abuf.tile([P, NCH, pstride], adt, tag=f"a{i}", name=f"a{i}")
                for i in range(n_img)
            ]
            for i in range(n_img):
                for ch in range(NCH):
                    zero_borders(a_ts[i], ch)
            # prewarm the collective path (first collective pays ~25us setup)
            wc_in = dramp.tile([P, 1], F32, tag="wc_in", name="wc_in")
            wc_out = dramp.tile(
                [n_cores * P, 1], F32, tag="wc_out", name="wc_out",
                addr_space="Shared" if n_cores > 4 else "Local",
            )
            nc.gpsimd.dma_start(out=wc_in, in_=cf_d[:, 0, 0:1])
            nc.gpsimd.collective_compute(
                "AllGather",
                Alu.bypass,
                replica_groups=[list(range(n_cores))],
                ins=[wc_in.opt()],
                outs=[wc_out.opt()],
            )
            stats1 = statsp.tile([P, NCH, n_img * nrb, 6], F32, tag="stats")
            with nc.named_scope("stage1"):
                for i in range(n_img):
                    for ch in range(NCH):
                        xt = xs.tile([P, hw], F32, tag="x", name="xt")
                        nq = 4 if h % 4 == 0 else 2
                        q = h // nq
                        sv = sign_view(a_ts[i], ch)
                        for hh in range(nq):
                            nc.sync.dma_start(
                                out=xt[:, hh * q * WID : (hh + 1) * q * WID]
                                .rearrange("p (r c) -> p r c", c=WID),
                                in_=x_d[
                                    i, ch * P : (ch + 1) * P,
                                    hh * q : (hh + 1) * q,
                                ],
                            )
                            nc.scalar.activation(
                                out=sv[:, hh * q : (hh + 1) * q, :],
                                in_=xt[:, hh * q * WID : (hh + 1) * q * WID]
                                .rearrange("p (r c) -> p r c", c=WID),
                                func=Act.Sign,
                            )
                # prewarm ACT tables during conv1 (needed in ar1/final)
                warm = small.tile([P, 1], F32, tag="warm")
                nc.scalar.activation(out=warm, in_=eps_t, func=Act.Sqrt)
                nc.scalar.activation(out=warm, in_=eps_t, func=Act.Identity,
                                     bias=eps_t)
                for i in range(n_img):
                    conv_one_img(w_t[0], a_ts[i], i, c_t[0], stats1)
            with nc.named_scope("ar1"):
                bn_coeffs(stats1, 0, 1, 4, stt1)

            # ================= stage 2 =================
            inner_t = {}
            stats2 = statsp.tile([P, NCH, n_img * nrb, 6], F32, tag="stats")
            with nc.named_scope("stage2"):
                for i in range(n_img):
                    for ch in range(NCH):
                        xt = xs.tile([P, hw], F32, tag="x", name="xt")
                        nq = 4 if (i == 0 and h % 4 == 0) else 1
                        q = h // nq
                        sv = sign_view(a_ts[i], ch)
                        half = h // 2
                        for hh in range(2):
                            nc.sync.dma_start(
                                out=xt[:, hh * half * WID : (hh + 1) * half * WID]
                                .rearrange("p (r c) -> p r c", c=WID),
                                in_=x_d[
                                    i, ch * P : (ch + 1) * P,
                                    hh * half : (hh + 1) * half,
                                ],
                            )
                        for hh in range(nq):
                            sl = slice(hh * q * WID, (hh + 1) * q * WID)
                            # t = c1 * A1 + x (in-place)
                            nc.vector.scalar_tensor_tensor(
                                out=xt[:, sl],
                                in0=c_t[0][(i, ch)][:, sl],
                                scalar=stt1[:, ch, 3:4],
                                in1=xt[:, sl],
                                op0=Alu.mult,
                                op1=Alu.add,
                            )
                            # a2 = sign(t + B1)
                            nc.scalar.activation(
                                out=sv[:, hh * q : (hh + 1) * q, :],
                                in_=xt[:, sl].rearrange("p (r c) -> p r c", c=WID),
                                func=Act.Sign,
                                bias=stt1[:, ch, 4:5],
                            )
                        # keep t as fp16, reusing the freed c1 slot
                        it = cbuf.tile(
                            [P, hw], F16, tag=f"c0_{i}_{ch}", name=f"inner_{i}_{ch}"
                        )
                        nc.vector.tensor_copy(out=it, in_=xt)
                        inner_t[(i, ch)] = it
                for i in range(n_img):
                    conv_one_img(w_t[1], a_ts[i], i, c_t[1], stats2)
            with nc.named_scope("ar2"):
                bn_coeffs(stats2, 2, 3, 5, stt2)
                nc.vector.tensor_add(b12, stt1[:, :, 4:5], stt2[:, :, 4:5])

            # ================= final =================
            with nc.named_scope("final"):
                for i in range(n_img):
                    for ch in range(NCH):
                        ft = xs.tile([P, hw], F16, tag="f16", name="ft", bufs=2)
                        xt = xs.tile([P, hw], F32, tag="x", name="xt")
                        # out = (c2*A2 + t) + (B1 + B2)
                        nc.vector.scalar_tensor_tensor(
                            out=ft, in0=c_t[1][(i, ch)],
                            scalar=stt2[:, ch, 3:4], in1=inner_t[(i, ch)],
                            op0=Alu.mult, op1=Alu.add,
                        )
                        nc.scalar.add(out=xt, in_=ft, add=b12[:, ch])
                        half = h // 2
                        for hh in range(2):
                            nc.sync.dma_start(
                                out=out_d[
                                    i, ch * P : (ch + 1) * P,
                                    hh * half : (hh + 1) * half,
                                ],
                                in_=xt[:, hh * half * WID : (hh + 1) * half * WID]
                                .rearrange("p (r c) -> p r c", c=WID),
                            )
    return nc


def prep_inputs(x, W1, gamma1, beta1, W2, gamma2, beta2, n_cores, n_img,
                use_dr=True):
    """Host-side prep: shard x, binarize/permute weights, pack BN coefs."""
    np_adt = NP_F8 if use_dr else NP_BF16

    def prep_w(Wm):
        Wm = np.asarray(Wm, np.float32)
        scale = np.float32(np.mean(np.abs(Wm)))
        s = np.sign(Wm).astype(np_adt)  # [co, ci, 3, 3]
        t = s.reshape(C, NCH, P, 3, 3)  # co, kch, p, dh, dw
        t = np.ascontiguousarray(t.transpose(2, 3, 4, 1, 0))  # p,dh,dw,kch,co
        return t.reshape(P, 9, NCH, C), scale

    w1b, s1 = prep_w(W1)
    w2b, s2 = prep_w(W2)
    g1 = np.asarray(gamma1, np.float32)
    b1 = np.asarray(beta1, np.float32)
    g2 = np.asarray(gamma2, np.float32)
    b2 = np.asarray(beta2, np.float32)
    coefs = np.zeros((P, NCH, 6), np.float32)
    coefs[:, :, 0] = (g1 * s1).reshape(NCH, P).T
    coefs[:, :, 1] = b1.reshape(NCH, P).T
    coefs[:, :, 2] = (g2 * s2).reshape(NCH, P).T
    coefs[:, :, 3] = b2.reshape(NCH, P).T
    coefs[:, :, 4] = np.float32(s1) ** 2
    coefs[:, :, 5] = np.float32(s2) ** 2

    x = np.asarray(x, np.float32)
    n, _, h, _ = x.shape
    assert n == n_cores * n_img
    xs = x.reshape(n_cores, n_img, C, h, WID)
    return [
        {
            "x": np.ascontiguousarray(xs[c]),
            "wb1": w1b,
            "wb2": w2b,
            "coefs": coefs,
        }
        for c in range(n_cores)
    ]


_NC_CACHE = {}


def _get_nc(n_img, h, n_cores, use_dr=True):
    key = (n_img, h, n_cores, use_dr)
    if key not in _NC_CACHE:
        nc = build_nc(n_img, h, n_cores, use_dr=use_dr)
        nc.compile()
        _NC_CACHE[key] = nc
    return _NC_CACHE[key]


_LAST_RESULT = None  # BassKernelResults of the most recent run (for test.py)


def kernel(x, W1, gamma1, beta1, W2, gamma2, beta2):
    global _LAST_RESULT
    x = np.asarray(x, np.float32)
    n_cores = 8
    n = x.shape[0]
    assert n % n_cores == 0
    n_img = n // n_cores
    h = x.shape[2]
    use_dr = True

    nc = _get_nc(n_img, h, n_cores, use_dr=use_dr)
    in_maps = prep_inputs(
        x, W1, gamma1, beta1, W2, gamma2, beta2, n_cores, n_img, use_dr=use_dr
    )
    res = bass_utils.run_bass_kernel_spmd(
        nc, in_maps, core_ids=list(range(n_cores)), trace=TRACE, **TRACE_KW
    )
    _LAST_RESULT = res
    out = np.concatenate([res.results[c]["out"] for c in range(n_cores)], axis=0)
    return out
